# revision 7
# baseline (speedup 1.0000x reference)
"""Trainium2 Bass kernel for nn_BatchIndexedMLP (segment_reduce).

Math (per batch row b):
  layer0: s[b, j] = sum_{i,k} x[b,i] * [batch_in_index[b,i,k] == j]   (j < 126;
          index 126 is the zero-pad row and is dropped)
          h1 = elu(s @ W0.T + b0)
  layer1: h2 = elu(h1 @ W1.T + b1)
  layer2: t[b, :] = h2 @ W2.T + b2  (t[b,126] = 0 pad)
          out[b, o] = sum_k t[b, batch_out_index[b,o,k]]

Kernel strategy (8 cores, data-parallel over B=1024 -> 128 rows/core):
  - batch rows on SBUF partitions
  - layer0 scatter: 126-bin DVE loop (is_equal mask + fused multiply-reduce)
  - matmuls on PE in feature-major (transposed) layout; elu composed as
    relu(y) + exp(min(y,0)) - 1 with the "-1" folded into the next layer's
    bias (host-adjusted biases)
  - layer2 gather: element-granular indirect DMA from a DRAM table
"""

import os
import sys

import numpy as np

for _p in ("/root/.axon_site/_ro/trn_rl_repo", "/opt/trn_rl_repo"):
    if os.path.isdir(_p) and _p not in sys.path:
        sys.path.append(_p)

import concourse.bass as bass
import concourse.mybir as mybir
from concourse import bacc
from concourse.bass_utils import run_bass_kernel_spmd
from concourse.masks import make_identity
from concourse.tile import TileContext

B, NIN, NOUT, K = 1024, 126, 126, 3
H1, H2 = 256, 256
NCORES = 8
P = B // NCORES          # 128 batch rows per core
Q = NIN * K              # 378 flattened (i, k) positions
NB = NIN + 1             # 127 bins incl. zero-pad bin
F32 = mybir.dt.float32
I32 = mybir.dt.int32
ALU = mybir.AluOpType
ACT = mybir.ActivationFunctionType


def _elu(nc, sb, psum_ap, out_ap, bias_ap, nbias_ap, scratch_shape):
    """out = relu(y) + exp(min(y, 0)) where y = psum + bias.

    This is elu(y) + 1; the caller folds the -1 into the next layer's bias.
    """
    rm = sb.tile(scratch_shape, F32, name="elu_rm", tag="elu_rm")
    e = sb.tile(scratch_shape, F32, name="elu_e", tag="elu_e")
    # relu(y)
    nc.scalar.activation(out=out_ap, in_=psum_ap, func=ACT.Relu, bias=bias_ap)
    # rm = relu(-y) = -min(y, 0)
    nc.scalar.activation(
        out=rm[:], in_=psum_ap, func=ACT.Relu, bias=nbias_ap, scale=-1.0
    )
    # e = exp(-rm) = exp(min(y, 0))
    nc.scalar.activation(out=e[:], in_=rm[:], func=ACT.Exp, scale=-1.0)
    nc.vector.tensor_tensor(out=out_ap, in0=out_ap, in1=e[:], op=ALU.add)


def build_kernel():
    nc = bacc.Bacc(None, target_bir_lowering=False)

    x_d = nc.dram_tensor("x", [P, NIN], F32, kind="ExternalInput")
    i1_d = nc.dram_tensor("idx1", [P, Q], I32, kind="ExternalInput")
    i2_d = nc.dram_tensor("idx2", [P, Q], I32, kind="ExternalInput")
    w0t_d = nc.dram_tensor("w0t", [NB, H1], F32, kind="ExternalInput")
    w1ta_d = nc.dram_tensor("w1ta", [128, H2], F32, kind="ExternalInput")
    w1tb_d = nc.dram_tensor("w1tb", [128, H2], F32, kind="ExternalInput")
    w2ta_d = nc.dram_tensor("w2ta", [128, NOUT], F32, kind="ExternalInput")
    w2tb_d = nc.dram_tensor("w2tb", [128, NOUT], F32, kind="ExternalInput")
    b0_d = [nc.dram_tensor(f"b0c{c}", [128, 1], F32, kind="ExternalInput") for c in range(2)]
    nb0_d = [nc.dram_tensor(f"nb0c{c}", [128, 1], F32, kind="ExternalInput") for c in range(2)]
    b1_d = [nc.dram_tensor(f"b1c{c}", [128, 1], F32, kind="ExternalInput") for c in range(2)]
    nb1_d = [nc.dram_tensor(f"nb1c{c}", [128, 1], F32, kind="ExternalInput") for c in range(2)]
    b2_d = nc.dram_tensor("b2c", [NOUT, 1], F32, kind="ExternalInput")
    out_d = nc.dram_tensor("out", [P, NOUT], F32, kind="ExternalOutput")

    with TileContext(nc) as tc:
        with (
            tc.tile_pool(name="sbuf", bufs=1) as sb,
            tc.tile_pool(name="psum", bufs=1, space="PSUM") as pp,
        ):
            # ---- loads ----
            x_sb = sb.tile([P, NIN], F32)
            i1_sb = sb.tile([P, Q], I32)
            i2_sb = sb.tile([P, Q], I32)
            w0t_sb = sb.tile([NB, H1], F32)
            w1_sb = [sb.tile([128, H2], F32, name=f"w1_{c}", tag=f"w1_{c}") for c in range(2)]
            w2_sb = [sb.tile([128, NOUT], F32, name=f"w2_{c}", tag=f"w2_{c}") for c in range(2)]
            b0_sb = [sb.tile([128, 1], F32, name=f"b0_{c}", tag=f"b0_{c}") for c in range(2)]
            nb0_sb = [sb.tile([128, 1], F32, name=f"nb0_{c}", tag=f"nb0_{c}") for c in range(2)]
            b1_sb = [sb.tile([128, 1], F32, name=f"b1_{c}", tag=f"b1_{c}") for c in range(2)]
            nb1_sb = [sb.tile([128, 1], F32, name=f"nb1_{c}", tag=f"nb1_{c}") for c in range(2)]
            b2_sb = sb.tile([NOUT, 1], F32)
            ident = sb.tile([128, 128], F32)

            nc.sync.dma_start(out=x_sb[:], in_=x_d[:, :])
            nc.sync.dma_start(out=i1_sb[:], in_=i1_d[:, :])
            nc.sync.dma_start(out=i2_sb[:], in_=i2_d[:, :])
            nc.sync.dma_start(out=w0t_sb[:], in_=w0t_d[:, :])
            nc.sync.dma_start(out=w1_sb[0][:], in_=w1ta_d[:, :])
            nc.sync.dma_start(out=w1_sb[1][:], in_=w1tb_d[:, :])
            nc.sync.dma_start(out=w2_sb[0][:], in_=w2ta_d[:, :])
            nc.sync.dma_start(out=w2_sb[1][:], in_=w2tb_d[:, :])
            for c in range(2):
                nc.sync.dma_start(out=b0_sb[c][:], in_=b0_d[c][:, :])
                nc.sync.dma_start(out=nb0_sb[c][:], in_=nb0_d[c][:, :])
                nc.sync.dma_start(out=b1_sb[c][:], in_=b1_d[c][:, :])
                nc.sync.dma_start(out=nb1_sb[c][:], in_=nb1_d[c][:, :])
            nc.sync.dma_start(out=b2_sb[:], in_=b2_d[:, :])
            make_identity(nc, ident[:])

            # ---- layer0 scatter: s[b, j] = sum_p x[b, p//3] * [idx1==j] ----
            # chunked 3-pass: mask (is_equal vs per-bin iota), product, reduce
            i1f = sb.tile([P, Q], F32)
            nc.vector.tensor_copy(out=i1f[:], in_=i1_sb[:])
            jall_i = sb.tile([P, NIN], I32)
            nc.gpsimd.iota(jall_i[:], [[1, NIN]], channel_multiplier=0)
            jall = sb.tile([P, NIN], F32)
            nc.vector.tensor_copy(out=jall[:], in_=jall_i[:])
            xrep = sb.tile([P, Q], F32)
            xrv = xrep[:].rearrange("p (i k) -> p i k", k=K)
            for k in range(K):
                nc.vector.tensor_copy(out=xrv[:, :, k], in_=x_sb[:])
            s_sb = sb.tile([P, NB], F32)
            nc.vector.memset(s_sb[:, NIN : NIN + 1], 0.0)
            JC = 21
            mask_big = sb.tile([P, JC * Q], F32)
            prod_big = sb.tile([P, JC * Q], F32)
            i1f_b = i1f[:].rearrange("p (j q) -> p j q", j=1).to_broadcast(
                [P, JC, Q]
            )
            xrep_b = xrep[:].rearrange("p (j q) -> p j q", j=1).to_broadcast(
                [P, JC, Q]
            )
            for c in range(NIN // JC):
                jb = (
                    jall[:, c * JC : (c + 1) * JC]
                    .rearrange("p (j q) -> p j q", q=1)
                    .to_broadcast([P, JC, Q])
                )
                mv = mask_big[:].rearrange("p (j q) -> p j q", j=JC)
                pv = prod_big[:].rearrange("p (j q) -> p j q", j=JC)
                nc.vector.tensor_tensor(out=mv, in0=i1f_b, in1=jb, op=ALU.is_equal)
                nc.vector.tensor_tensor(out=pv, in0=mv, in1=xrep_b, op=ALU.mult)
                nc.vector.tensor_reduce(
                    out=s_sb[:, c * JC : (c + 1) * JC],
                    in_=pv,
                    axis=mybir.AxisListType.X,
                    op=ALU.add,
                )

            # ---- transpose s -> sT (bins on partitions) ----
            sT_ps = pp.tile([NB, P], F32)
            nc.tensor.transpose(out=sT_ps[:], in_=s_sb[:], identity=ident[:])
            sT_sb = sb.tile([NB, P], F32)
            nc.scalar.copy(out=sT_sb[:], in_=sT_ps[:])

            # ---- MM1: hT[f, b] = sum_j w0t[j, f] * sT[j, b], f in 2 chunks ----
            hT_sb = sb.tile([128, 2 * P], F32)
            for c in range(2):
                h_ps = pp.tile([128, P], F32, name=f"h_ps{c}", tag=f"h_ps{c}")
                nc.tensor.matmul(
                    out=h_ps[:],
                    lhsT=w0t_sb[:, c * 128 : (c + 1) * 128],
                    rhs=sT_sb[:],
                    start=True,
                    stop=True,
                )
                _elu(
                    nc, sb, h_ps[:],
                    hT_sb[:, c * P : (c + 1) * P],
                    b0_sb[c][:],
                    nb0_sb[c][:],
                    [128, P],
                )

            # ---- MM2: h2T[o, b] = sum_f w1t[f, o] * hT[f, b] ----
            h2T_sb = sb.tile([128, 2 * P], F32)
            for d in range(2):
                h2_ps = pp.tile([128, P], F32, name=f"h2_ps{d}", tag=f"h2_ps{d}")
                for c in range(2):
                    nc.tensor.matmul(
                        out=h2_ps[:],
                        lhsT=w1_sb[c][:, d * 128 : (d + 1) * 128],
                        rhs=hT_sb[:, c * P : (c + 1) * P],
                        start=(c == 0),
                        stop=(c == 1),
                    )
                _elu(
                    nc, sb, h2_ps[:],
                    h2T_sb[:, d * P : (d + 1) * P],
                    b1_sb[d][:],
                    nb1_sb[d][:],
                    [128, P],
                )

            # ---- MM3: tT[j, b] = sum_o w2t[o, j] * h2T[o, b] + b2adj[j] ----
            t_ps = pp.tile([NOUT, P], F32)
            for d in range(2):
                nc.tensor.matmul(
                    out=t_ps[:],
                    lhsT=w2_sb[d][:],
                    rhs=h2T_sb[:, d * P : (d + 1) * P],
                    start=(d == 0),
                    stop=(d == 1),
                )
            t_sb = sb.tile([NB, P], F32)
            nc.vector.memset(t_sb[:], 0.0)
            nc.scalar.activation(
                out=t_sb[:NOUT, :], in_=t_ps[:], func=ACT.Identity, bias=b2_sb[:]
            )

            # ---- transpose t -> batch-major (128, 127) ----
            tbm_ps = pp.tile([P, NB], F32)
            nc.tensor.transpose(
                out=tbm_ps[:], in_=t_sb[:], identity=ident[:NB, :NB]
            )
            t_bm = sb.tile([P, NB], F32)
            nc.scalar.copy(out=t_bm[:], in_=tbm_ps[:])

            # ---- layer2 gather as fused mask loop ----
            # g[b, q] = t[b, idx2[b, q]] = sum_j [idx2==j] * t_bm[b, j]
            i2f = sb.tile([P, Q], F32)
            nc.vector.tensor_copy(out=i2f[:], in_=i2_sb[:])
            acc = sb.tile([P, Q], F32)
            nc.vector.memset(acc[:], 0.0)
            tmp2 = sb.tile([P, Q], F32)
            for j in range(NOUT):
                nc.vector.tensor_scalar(
                    out=tmp2[:],
                    in0=i2f[:],
                    scalar1=float(j),
                    scalar2=t_bm[:, j : j + 1],
                    op0=ALU.is_equal,
                    op1=ALU.mult,
                )
                nc.vector.tensor_tensor(
                    out=acc[:], in0=acc[:], in1=tmp2[:], op=ALU.add
                )
            gv = acc[:].rearrange("p (o k) -> p o k", k=K)
            o_sb = sb.tile([P, NOUT], F32)
            nc.vector.tensor_tensor(
                out=o_sb[:], in0=gv[:, :, 0], in1=gv[:, :, 1], op=ALU.add
            )
            nc.vector.tensor_tensor(
                out=o_sb[:], in0=o_sb[:], in1=gv[:, :, 2], op=ALU.add
            )
            nc.sync.dma_start(out=out_d[:, :], in_=o_sb[:])

    nc.finalize()
    return nc


_NC_CACHE = None


def _get_nc():
    global _NC_CACHE
    if _NC_CACHE is None:
        _NC_CACHE = build_kernel()
    return _NC_CACHE


def _prep_in_maps(x, W0, b0, W1, b1, W2, b2, batch_in_index, batch_out_index):
    x = np.ascontiguousarray(x, dtype=np.float32)
    idx1 = np.ascontiguousarray(
        batch_in_index.reshape(B, Q), dtype=np.int32
    )
    idx2 = np.ascontiguousarray(
        batch_out_index.reshape(B, Q), dtype=np.int32
    )
    w0t = np.concatenate(
        [np.asarray(W0, np.float32).T, np.zeros((1, H1), np.float32)], axis=0
    )  # (127, 256)
    w1t = np.ascontiguousarray(np.asarray(W1, np.float32).T)  # (256, 256)
    w2t = np.ascontiguousarray(np.asarray(W2, np.float32).T)  # (256, 126)
    b0c = np.asarray(b0, np.float32).reshape(H1, 1)
    # fold the elu "-1" shift into downstream biases
    b1adj = (np.asarray(b1, np.float32) - np.asarray(W1, np.float32).sum(1))
    b2adj = (np.asarray(b2, np.float32) - np.asarray(W2, np.float32).sum(1))
    b1c = b1adj.reshape(H2, 1).astype(np.float32)
    b2c = b2adj.reshape(NOUT, 1).astype(np.float32)
    shared = dict(
        w0t=np.ascontiguousarray(w0t),
        w1ta=np.ascontiguousarray(w1t[:128]),
        w1tb=np.ascontiguousarray(w1t[128:]),
        w2ta=np.ascontiguousarray(w2t[:128]),
        w2tb=np.ascontiguousarray(w2t[128:]),
        b0c0=np.ascontiguousarray(b0c[:128]),
        b0c1=np.ascontiguousarray(b0c[128:]),
        nb0c0=np.ascontiguousarray(-b0c[:128]),
        nb0c1=np.ascontiguousarray(-b0c[128:]),
        b1c0=np.ascontiguousarray(b1c[:128]),
        b1c1=np.ascontiguousarray(b1c[128:]),
        nb1c0=np.ascontiguousarray(-b1c[:128]),
        nb1c1=np.ascontiguousarray(-b1c[128:]),
        b2c=b2c,
    )
    in_maps = []
    for c in range(NCORES):
        rows = slice(c * P, (c + 1) * P)
        in_maps.append(
            dict(
                x=np.ascontiguousarray(x[rows]),
                idx1=np.ascontiguousarray(idx1[rows]),
                idx2=np.ascontiguousarray(idx2[rows]),
                **shared,
            )
        )
    return in_maps


def kernel(**inputs) -> np.ndarray:
    nc = _get_nc()
    in_maps = _prep_in_maps(**inputs)
    res = run_bass_kernel_spmd(nc, in_maps, core_ids=list(range(NCORES)))
    return np.concatenate([r["out"] for r in res.results], axis=0)


if __name__ == "__main__":
    rng = np.random.default_rng(0)
    ins = dict(
        x=rng.standard_normal((B, NIN)).astype(np.float32),
        W0=rng.standard_normal((H1, NIN)).astype(np.float32) * 0.09,
        b0=rng.standard_normal((H1,)).astype(np.float32) * 0.09,
        W1=rng.standard_normal((H2, H1)).astype(np.float32) * 0.0625,
        b1=rng.standard_normal((H2,)).astype(np.float32) * 0.0625,
        W2=rng.standard_normal((NOUT, H2)).astype(np.float32) * 0.0625,
        b2=rng.standard_normal((NOUT,)).astype(np.float32) * 0.0625,
        batch_in_index=rng.integers(0, NIN + 1, (B, NIN, K)).astype(np.int32),
        batch_out_index=rng.integers(0, NOUT + 1, (B, NOUT, K)).astype(np.int32),
    )
    out = kernel(**ins)
    print("out", out.shape, out.dtype, np.abs(out).mean())
